# revision 3
# baseline (speedup 1.0000x reference)
"""Trainium2 Bass kernel v2 for nn_Attention_67336497266780.

Single-head attention, B=8 S=2048 E=1024 H=64, data-parallel over batch:
each of the 8 NeuronCores computes one batch element end to end.

v2 changes vs baseline:
  - X path in fp16 (SWDGE cast-DMA from f32 DRAM): PE transposes at
    1 cyc/row (vs 2 for f32), DVE 2x-mode PSUM->SBUF copies, FWL on the
    projection weight loads.
  - Quad granularity (512 rows per DMA = 1 MB) for the input stream.
  - Projection col-packed 2x: two 64-row output chains at col groups
    0 and 64 of the PE array run concurrently on HW.
  - Score matmuls interleave the two row-packed k-tiles so consecutive
    matmuls target different PE row groups (concurrent on HW).
"""
import sys

sys.path.insert(0, "/opt/trn_rl_repo")

from contextlib import ExitStack

import numpy as np

import concourse.bass as bass
import concourse.mybir as mybir
import concourse.tile as tile
from concourse import bacc
from concourse.masks import make_identity

F32 = mybir.dt.float32
F32R = mybir.dt.float32r
FP16 = mybir.dt.float16

B = 8
P = 128
S = 2048
E = 1024
H = 64
EC = E // P          # 8 e-chunks
ST = S // P          # 16 s-tiles
NQUAD = ST // 4      # 4 quads (512 rows each)
QC = 4               # q chunks for scores/pv
QCHUNK = S // QC     # 512


def build(
    x_dt=FP16,
    scores_dt=FP16,
    pv_dt=FP16,
    debug=False,
    repeat=0,
    ablate="",   # "" | "dma" | "sweep" | "noattend" | "nopv"
    dma_engine="gpsimd",
    tmode="transpose",  # "matmul": x-transposes as regular matmul vs identity
                     # (exact for fp16 data, HAM-warm, FWL); "transpose":
                     # PE transpose-mode
):
    nc = bacc.Bacc("TRN2", target_bir_lowering=False, debug=debug)

    xq_ext = nc.dram_tensor("query", [S, E], F32, kind="ExternalInput")
    xk_ext = nc.dram_tensor("key", [S, E], F32, kind="ExternalInput")
    xv_ext = nc.dram_tensor("value", [S, E], F32, kind="ExternalInput")
    wq_ext = nc.dram_tensor("Wq", [E, H], F32, kind="ExternalInput")
    wk_ext = nc.dram_tensor("Wk", [E, H], F32, kind="ExternalInput")
    wv_ext = nc.dram_tensor("Wv", [E, H], F32, kind="ExternalInput")
    bq_ext = nc.dram_tensor("bq", [H], F32, kind="ExternalInput")
    bk_ext = nc.dram_tensor("bk", [H], F32, kind="ExternalInput")
    bv_ext = nc.dram_tensor("bv", [H], F32, kind="ExternalInput")
    out_ext = nc.dram_tensor("out", [S, H], F32, kind="ExternalOutput")

    ctx = ExitStack()
    with tile.TileContext(nc) as tc, ctx:
        const = ctx.enter_context(tc.tile_pool(name="const", bufs=1))
        persist = ctx.enter_context(tc.tile_pool(name="persist", bufs=1))
        xpool = ctx.enter_context(tc.tile_pool(name="xpool", bufs=3))
        xtpool = ctx.enter_context(tc.tile_pool(name="xtpool", bufs=3))
        # transposes get their own psum pool: x_t buffers (which gate the
        # DMA stream) are released by transposes alone, so these must never
        # queue behind tail PV/epilogue psum tiles
        ps_tp = ctx.enter_context(tc.tile_pool(name="ps_tp", bufs=2, space="PSUM"))
        ps_work = ctx.enter_context(tc.tile_pool(name="ps_work", bufs=2, space="PSUM"))
        ps_sc = ctx.enter_context(tc.tile_pool(name="ps_sc", bufs=2, space="PSUM"))

        if repeat:
            hint = ((mybir.EngineType.Pool,) if ablate == "dma" else
                    (mybir.EngineType.PE, mybir.EngineType.DVE))
            loop_cm = tc.For_i(0, repeat, 1, hint_engines=hint)
        else:
            import contextlib
            loop_cm = contextlib.nullcontext()

        # ---- constants (outside timing loop) ----
        ident = const.tile([P, P], x_dt, name="ident")
        make_identity(nc, ident)
        ident_f32 = const.tile([P, P], F32, name="ident_f32")
        make_identity(nc, ident_f32)

        w_sb = {}
        b_sb = {}
        for name, wext, bext in (("q", wq_ext, bq_ext), ("k", wk_ext, bk_ext),
                                 ("v", wv_ext, bv_ext)):
            w = const.tile([P, EC, H], x_dt, name=f"w{name}")
            nc.gpsimd.dma_start(w[:], wext.rearrange("(o p) h -> p o h", p=P))
            w_sb[name] = w
            b = const.tile([H, 1], F32, name=f"b{name}")
            nc.scalar.dma_start(b[:], bext[:].unsqueeze(1))
            b_sb[name] = b

        # qt/kt duplicated across both partition halves for score rowpacking
        qt_sb = persist.tile([P, S], scores_dt, name="qt")
        kt_sb = persist.tile([P, S], scores_dt, name="kt")
        vt_sb = persist.tile([H, S], F32, name="vt")
        vp_sb = persist.tile([P, ST, H + 1], pv_dt, name="vprime")
        exp_all = persist.tile([P, ST, S], pv_dt, name="exp_all")

        copy_ctr = [0]

        def copy_op(out, in_, period=3):
            # fp16 copies go to DVE (2x perf mode); spread 1/period to ACT
            i = copy_ctr[0]
            copy_ctr[0] += 1
            if i % period == period - 1:
                nc.scalar.copy(out=out, in_=in_)
            else:
                nc.vector.tensor_copy(out=out, in_=in_)

        with loop_cm:
            if ablate == "dma":
                for quad in range(NQUAD):
                    for xext in (xq_ext, xk_ext, xv_ext):
                        s0 = quad * 4 * P
                        src = xext[s0:s0 + 4 * P, :].rearrange(
                            "(t p) e -> p t e", p=P)
                        if dma_engine == "gpsimd":
                            x_t = xpool.tile([P, 4, E], x_dt, tag="x")
                            nc.gpsimd.dma_start(x_t[:], src)
                        else:
                            x_t = xpool.tile([P, 4, E], F32, tag="xf")
                            nc.sync.dma_start(x_t[:], src)
            do_full = ablate != "dma"
            nc.vector.memset(vp_sb[:, :, H:H + 1], 1.0)

            def sweep_quad(xext, quad, tag):
                """Load + transpose + project 512 rows (4 s-tiles) of one
                input tensor. Projection col-packed 2x: pairs (A, B) of
                256 rows each accumulate into PSUM partitions 0-63 /
                64-127 concurrently."""
                s0 = quad * 4 * P
                x_t = xpool.tile([P, 4, E], x_dt, tag="x")
                src = xext[s0:s0 + 4 * P, :].rearrange("(t p) e -> p t e", p=P)
                nc.gpsimd.dma_start(x_t[:], src)  # SWDGE cast f32->fp16
                xt_t = xtpool.tile([P, EC, 4 * P], x_dt, tag="xt")
                if tmode == "matmul":
                    # regular matmul vs fp16 identity: exact transpose for
                    # fp16 data, HAM-warm, FWL; psum out must be f32 (1 bank
                    # per c-chunk), so copies are f32-sourced
                    for c in range(EC):
                        t_ps = ps_tp.tile([P, 4 * P], F32, tag="ps")
                        for j in range(4):
                            nc.tensor.matmul(
                                t_ps[:, j * P:(j + 1) * P],
                                lhsT=x_t[:, j, c * P:(c + 1) * P],
                                rhs=ident[:],
                                start=True, stop=True)
                        copy_op(xt_t[:, c], t_ps[:],
                                period=(2 if tag == "q" else 3))
                else:
                    for c2 in range(EC // 2):
                        t_ps = ps_tp.tile([P, 2, 4 * P], x_dt, tag="ps")
                        for cc in range(2):
                            c = 2 * c2 + cc
                            for j in range(4):
                                nc.tensor.transpose(
                                    t_ps[:, cc, j * P:(j + 1) * P],
                                    x_t[:, j, c * P:(c + 1) * P],
                                    ident)
                        # q sweep: ACT is idle early -> give it half the
                        # copies; k/v sweeps run while ACT does exp -> keep
                        # copies on DVE
                        copy_op(xt_t[:, 2 * c2:2 * c2 + 2], t_ps[:],
                                period=(2 if tag == "q" else 1000))
                # col-packed: pair A -> out partitions 0-63 (col grp 0),
                # pair B -> 64-127 (col grp 64); interleaved emission makes
                # them concurrent on HW. Separate PSUM tiles (banks) keep the
                # two accumulation groups' has_written state independent.
                psA = ps_work.tile([P, 2 * P], F32, tag="ps", name="projA")
                psB = ps_work.tile([P, 2 * P], F32, tag="ps", name="projB")
                w = w_sb[tag]
                for c in range(EC):
                    nc.tensor.matmul(
                        psA[0:H], lhsT=w[:, c], rhs=xt_t[:, c, 0:2 * P],
                        start=(c == 0), stop=(c == EC - 1))
                    nc.tensor.matmul(
                        psB[H:2 * H], lhsT=w[:, c],
                        rhs=xt_t[:, c, 2 * P:4 * P],
                        start=(c == 0), stop=(c == EC - 1))
                bias = b_sb[tag]
                slA = slice(s0, s0 + 2 * P)
                slB = slice(s0 + 2 * P, s0 + 4 * P)
                if tag == "v":
                    for sl_, src_ in ((slA, psA[0:H]), (slB, psB[H:2 * H])):
                        nc.vector.tensor_scalar(
                            out=vt_sb[:, sl_], in0=src_,
                            scalar1=bias, scalar2=None, op0=mybir.AluOpType.add)
                else:
                    dst = qt_sb if tag == "q" else kt_sb
                    for sl_, src_ in ((slA, psA[0:H]), (slB, psB[H:2 * H])):
                        nc.vector.tensor_scalar(
                            out=dst[0:H, sl_], in0=src_, scalar1=bias,
                            scalar2=None, op0=mybir.AluOpType.add)
                        # rowpack duplicate: fp16 SBUF->SBUF copy runs in
                        # DVE 4x mode
                        nc.vector.tensor_copy(out=dst[H:2 * H, sl_],
                                              in_=dst[0:H, sl_])

            def build_vprime(m):
                t_ps = ps_tp.tile([P, H], F32, tag="ps")
                nc.tensor.transpose(
                    t_ps[:], vt_sb[:, m * P:(m + 1) * P],
                    ident_f32[:H, :H])
                nc.vector.tensor_copy(out=vp_sb[:, m, 0:H], in_=t_ps[:])

            def attend_kpair(p):
                """scores+exp for k-tiles m=2p, 2p+1 (row-packed).
                Interleave the two k-tiles so consecutive matmuls hit
                different PE row groups (concurrent on HW)."""
                ms = (2 * p, 2 * p + 1)
                sc = {}
                for idx, m in enumerate(ms):
                    sc[m] = ps_sc.tile([P, 2 * QCHUNK], F32, tag="sc",
                                       name=f"sc_{idx}")
                for qq in range(QC // 2):
                    for half in range(2):
                        qc = 2 * qq + half
                        for idx, m in enumerate(ms):
                            row0 = idx * H
                            nc.tensor.matmul(
                                sc[m][:, half * QCHUNK:(half + 1) * QCHUNK],
                                lhsT=kt_sb[row0:row0 + H, m * P:(m + 1) * P],
                                rhs=qt_sb[row0:row0 + H,
                                          qc * QCHUNK:(qc + 1) * QCHUNK],
                                start=True, stop=True)
                    for idx, m in enumerate(ms):
                        nc.scalar.activation(
                            exp_all[:, m, qq * 2 * QCHUNK:(qq + 1) * 2 * QCHUNK],
                            sc[m][:],
                            mybir.ActivationFunctionType.Exp, scale=0.125)

            # ---------------- phase 1a: full Xq sweep ----------------
            if do_full:
                for quad in range(NQUAD):
                    sweep_quad(xq_ext, quad, "q")

            outA_sb = persist.tile([H + 1, S], F32, name="outA_sb")

            def pv_half(qc, m_lo, m_hi, dst, accum_from=None):
                sl = slice(qc * QCHUNK, (qc + 1) * QCHUNK)
                pv_full = ps_work.tile([P, QCHUNK], F32, tag="ps")
                pv_ps = pv_full[:H + 1]
                for m in range(m_lo, m_hi):
                    nc.tensor.matmul(
                        pv_ps, lhsT=vp_sb[:, m],
                        rhs=exp_all[:, m, sl],
                        start=(m == m_lo), stop=(m == m_hi - 1))
                if accum_from is None:
                    copy_op(dst[:, sl], pv_ps)
                else:
                    nc.vector.tensor_tensor(
                        dst[:, sl], pv_ps, accum_from[:, sl],
                        mybir.AluOpType.add)

            # ------- phase 1b: K stream with attends; then V stream -------
            # k before v: all scores/exp work is generated while the v
            # stream still has DMA to hide it under, and the last quad's
            # attends no longer land in the tail.
            for quad in range(NQUAD if do_full else 0):
                sweep_quad(xk_ext, quad, "k")
                if ablate in ("sweep", "noattend"):
                    continue
                attend_kpair(2 * quad)
                attend_kpair(2 * quad + 1)
            for quad in range(NQUAD if do_full else 0):
                sweep_quad(xv_ext, quad, "v")
                if ablate == "sweep":
                    continue
                for t in range(4):
                    build_vprime(4 * quad + t)
                if quad == 2 and ablate not in ("noattend", "nopv"):
                    # vp m0..7 ready after v-quad 1; exps all done in the k
                    # phase: fold in first-half PV while v-quad 3 streams
                    for qc in range(QC):
                        pv_half(qc, 0, ST // 2, outA_sb)

            # ------------- phase 3: PV-B + epilogue per q-chunk -------------
            outT_sb = persist.tile([H + 1, S], F32, name="outT_sb")
            out_sb = persist.tile([P, ST, H], F32, name="out_sb")
            rc_sb = persist.tile([P, ST], F32, name="rc")
            for qc in range(QC if (do_full and ablate not in ("sweep", "noattend", "nopv")) else 0):
                sl = slice(qc * QCHUNK, (qc + 1) * QCHUNK)
                pv_half(qc, ST // 2, ST, outT_sb, accum_from=outA_sb)
                for mm in range(qc * QC, (qc + 1) * QC):
                    o_ps = ps_work.tile([P, H + 1], F32, tag="ps")
                    nc.tensor.transpose(
                        o_ps[:], outT_sb[:, mm * P:(mm + 1) * P],
                        ident_f32[:H + 1, :H + 1])
                    nc.vector.reciprocal(rc_sb[:, mm:mm + 1], o_ps[:, H:H + 1])
                    nc.vector.tensor_scalar(
                        out=out_sb[:, mm], in0=o_ps[:, 0:H],
                        scalar1=rc_sb[:, mm:mm + 1],
                        scalar2=None, op0=mybir.AluOpType.mult)
                if ablate != "noout":
                    nc.scalar.dma_start(
                        out_ext[qc * QC * P:(qc + 1) * QC * P, :].rearrange(
                            "(t p) h -> p t h", p=P),
                        out_sb[:, qc * QC:(qc + 1) * QC])

    nc.compile()
    return nc


_CACHE = {}


def _get_runner():
    if "runner" in _CACHE:
        return _CACHE["runner"]

    import functools
    import traceback

    import jax
    from jax.experimental.shard_map import shard_map
    from jax.sharding import Mesh, PartitionSpec

    from concourse import bass2jax
    from concourse.bass2jax import _bass_exec_p, partition_id_tensor

    bass2jax.install_neuronx_cc_hook()
    import libneuronxla
    hook = libneuronxla.neuronx_cc
    if not getattr(hook, "_verbose_wrapped", False):
        @functools.wraps(hook)
        def wrapped(*a, **k):
            try:
                return hook(*a, **k)
            except BaseException:
                traceback.print_exc()
                sys.stderr.flush()
                raise
        wrapped._verbose_wrapped = True
        libneuronxla.neuronx_cc = wrapped

    nc = build()

    partition_name = nc.partition_id_tensor.name if nc.partition_id_tensor else None
    in_names, out_names, out_avals, zero_outs = [], [], [], []
    for alloc in nc.m.functions[0].allocations:
        if not isinstance(alloc, mybir.MemoryLocationSet):
            continue
        name = alloc.memorylocations[0].name
        if alloc.kind == "ExternalInput":
            if name != partition_name:
                in_names.append(name)
        elif alloc.kind == "ExternalOutput":
            out_names.append(name)
            shape = tuple(alloc.tensor_shape)
            dtype = mybir.dt.np(alloc.dtype)
            out_avals.append(jax.core.ShapedArray(shape, dtype))
            zero_outs.append(np.zeros(shape, dtype))
    n_params = len(in_names)
    n_outs = len(out_avals)
    all_in_names = list(in_names) + out_names
    if partition_name is not None:
        all_in_names.append(partition_name)
    donate = tuple(range(n_params, n_params + n_outs))

    def _body(*args):
        operands = list(args)
        if partition_name is not None:
            operands.append(partition_id_tensor())
        outs = _bass_exec_p.bind(
            *operands,
            out_avals=tuple(out_avals),
            in_names=tuple(all_in_names),
            out_names=tuple(out_names),
            lowering_input_output_aliases=(),
            sim_require_finite=True,
            sim_require_nnan=True,
            nc=nc,
        )
        return tuple(outs)

    devices = jax.devices()[:B]
    mesh = Mesh(np.asarray(devices), ("core",))
    in_specs = (PartitionSpec("core"),) * (n_params + n_outs)
    out_specs = (PartitionSpec("core"),) * len(out_names)
    sharded = jax.jit(
        shard_map(_body, mesh=mesh, in_specs=in_specs,
                  out_specs=out_specs, check_rep=False),
        donate_argnums=donate, keep_unused=True)

    runner = {
        "sharded": sharded, "in_names": in_names, "out_names": out_names,
        "out_avals": out_avals, "zero_outs": zero_outs,
    }
    _CACHE["runner"] = runner
    return runner


def kernel(**inputs):
    r = _get_runner()
    per_core = {"query", "key", "value"}

    concat_in = []
    for name in r["in_names"]:
        arr = np.ascontiguousarray(np.asarray(inputs[name], dtype=np.float32))
        if name in per_core:
            concat_in.append(arr.reshape(B * S, E))
        else:
            concat_in.append(np.concatenate([arr] * B, axis=0))
    concat_zeros = [
        np.zeros((B * z.shape[0], *z.shape[1:]), z.dtype) for z in r["zero_outs"]
    ]
    out_arrs = r["sharded"](*concat_in, *concat_zeros)
    (aval,) = r["out_avals"]
    out = np.asarray(out_arrs[0]).reshape(B, *aval.shape)
    return out.astype(np.float32, copy=False)


if __name__ == "__main__":
    rng = np.random.default_rng(0)
    fake = {
        "query": rng.standard_normal((B, S, E), dtype=np.float32),
        "key": rng.standard_normal((B, S, E), dtype=np.float32),
        "value": rng.standard_normal((B, S, E), dtype=np.float32),
        "Wq": rng.standard_normal((E, H), dtype=np.float32) / 32,
        "bq": np.zeros(H, np.float32),
        "Wk": rng.standard_normal((E, H), dtype=np.float32) / 32,
        "bk": np.zeros(H, np.float32),
        "Wv": rng.standard_normal((E, H), dtype=np.float32) / 32,
        "bv": np.zeros(H, np.float32),
    }
    out = kernel(**fake)
    print("kernel out:", out.shape, out.dtype, float(out[0, 0, 0]))



# revision 4
# speedup vs baseline: 1.1241x; 1.1241x over previous
"""Trainium2 Bass kernel v2 for nn_Attention_67336497266780.

Single-head attention, B=8 S=2048 E=1024 H=64, data-parallel over batch:
each of the 8 NeuronCores computes one batch element end to end.

v2 changes vs baseline:
  - X path in fp16 (SWDGE cast-DMA from f32 DRAM): PE transposes at
    1 cyc/row (vs 2 for f32), DVE 2x-mode PSUM->SBUF copies, FWL on the
    projection weight loads.
  - Quad granularity (512 rows per DMA = 1 MB) for the input stream.
  - Projection col-packed 2x: two 64-row output chains at col groups
    0 and 64 of the PE array run concurrently on HW.
  - Score matmuls interleave the two row-packed k-tiles so consecutive
    matmuls target different PE row groups (concurrent on HW).
"""
import sys

sys.path.insert(0, "/opt/trn_rl_repo")

from contextlib import ExitStack

import numpy as np

import concourse.bass as bass
import concourse.mybir as mybir
import concourse.tile as tile
from concourse import bacc
from concourse.masks import make_identity

F32 = mybir.dt.float32
F32R = mybir.dt.float32r
FP16 = mybir.dt.float16

B = 8
P = 128
S = 2048
E = 1024
H = 64
EC = E // P          # 8 e-chunks
ST = S // P          # 16 s-tiles
NQUAD = ST // 4      # 4 quads (512 rows each)
QC = 4               # q chunks for scores/pv
QCHUNK = S // QC     # 512


def build(
    x_dt=FP16,
    scores_dt=FP16,
    pv_dt=FP16,
    debug=False,
    repeat=0,
    ablate="",   # "" | "dma" | "sweep" | "noattend" | "nopv"
    dma_engine="gpsimd",
    order="kv",      # "kv": q,(k,v)-interleaved | "ktv": q,k+attend,v+vprime
    tmode="transpose",  # "matmul": x-transposes as regular matmul vs identity
                     # (exact for fp16 data, HAM-warm, FWL); "transpose":
                     # PE transpose-mode
):
    nc = bacc.Bacc("TRN2", target_bir_lowering=False, debug=debug)

    xq_ext = nc.dram_tensor("query", [S, E], F32, kind="ExternalInput")
    xk_ext = nc.dram_tensor("key", [S, E], F32, kind="ExternalInput")
    xv_ext = nc.dram_tensor("value", [S, E], F32, kind="ExternalInput")
    wq_ext = nc.dram_tensor("Wq", [E, H], F32, kind="ExternalInput")
    wk_ext = nc.dram_tensor("Wk", [E, H], F32, kind="ExternalInput")
    wv_ext = nc.dram_tensor("Wv", [E, H], F32, kind="ExternalInput")
    bq_ext = nc.dram_tensor("bq", [H], F32, kind="ExternalInput")
    bk_ext = nc.dram_tensor("bk", [H], F32, kind="ExternalInput")
    bv_ext = nc.dram_tensor("bv", [H], F32, kind="ExternalInput")
    out_ext = nc.dram_tensor("out", [S, H], F32, kind="ExternalOutput")

    ctx = ExitStack()
    with tile.TileContext(nc) as tc, ctx:
        const = ctx.enter_context(tc.tile_pool(name="const", bufs=1))
        persist = ctx.enter_context(tc.tile_pool(name="persist", bufs=1))
        xpool = ctx.enter_context(tc.tile_pool(name="xpool", bufs=3))
        xtpool = ctx.enter_context(tc.tile_pool(name="xtpool", bufs=3))
        ps_work = ctx.enter_context(tc.tile_pool(name="ps_work", bufs=4, space="PSUM"))
        ps_sc = ctx.enter_context(tc.tile_pool(name="ps_sc", bufs=2, space="PSUM"))

        if repeat:
            hint = ((mybir.EngineType.Pool,) if ablate == "dma" else
                    (mybir.EngineType.PE, mybir.EngineType.DVE))
            loop_cm = tc.For_i(0, repeat, 1, hint_engines=hint)
        else:
            import contextlib
            loop_cm = contextlib.nullcontext()

        # ---- constants (outside timing loop) ----
        ident = const.tile([P, P], x_dt, name="ident")
        make_identity(nc, ident)
        ident_f32 = const.tile([P, P], F32, name="ident_f32")
        make_identity(nc, ident_f32)

        w_sb = {}
        b_sb = {}
        for name, wext, bext in (("q", wq_ext, bq_ext), ("k", wk_ext, bk_ext),
                                 ("v", wv_ext, bv_ext)):
            w = const.tile([P, EC, H], x_dt, name=f"w{name}")
            nc.gpsimd.dma_start(w[:], wext.rearrange("(o p) h -> p o h", p=P))
            w_sb[name] = w
            b = const.tile([H, 1], F32, name=f"b{name}")
            nc.scalar.dma_start(b[:], bext[:].unsqueeze(1))
            b_sb[name] = b

        # qt/kt duplicated across both partition halves for score rowpacking
        qt_sb = persist.tile([P, S], scores_dt, name="qt")
        kt_sb = persist.tile([P, S], scores_dt, name="kt")
        vt_sb = persist.tile([H, S], F32, name="vt")
        vp_sb = persist.tile([P, ST, H + 1], pv_dt, name="vprime")
        exp_all = persist.tile([P, ST, S], pv_dt, name="exp_all")

        copy_ctr = [0]

        def copy_op(out, in_, period=3):
            # fp16 copies go to DVE (2x perf mode); spread 1/period to ACT
            i = copy_ctr[0]
            copy_ctr[0] += 1
            if i % period == period - 1:
                nc.scalar.copy(out=out, in_=in_)
            else:
                nc.vector.tensor_copy(out=out, in_=in_)

        with loop_cm:
            if ablate == "dma":
                for quad in range(NQUAD):
                    for xext in (xq_ext, xk_ext, xv_ext):
                        s0 = quad * 4 * P
                        src = xext[s0:s0 + 4 * P, :].rearrange(
                            "(t p) e -> p t e", p=P)
                        if dma_engine == "gpsimd":
                            x_t = xpool.tile([P, 4, E], x_dt, tag="x")
                            nc.gpsimd.dma_start(x_t[:], src)
                        else:
                            x_t = xpool.tile([P, 4, E], F32, tag="xf")
                            nc.sync.dma_start(x_t[:], src)
            do_full = ablate != "dma"
            nc.vector.memset(vp_sb[:, :, H:H + 1], 1.0)

            def sweep_quad(xext, quad, tag):
                """Load + transpose + project 512 rows (4 s-tiles) of one
                input tensor. Projection col-packed 2x: pairs (A, B) of
                256 rows each accumulate into PSUM partitions 0-63 /
                64-127 concurrently."""
                s0 = quad * 4 * P
                x_t = xpool.tile([P, 4, E], x_dt, tag="x")
                src = xext[s0:s0 + 4 * P, :].rearrange("(t p) e -> p t e", p=P)
                nc.gpsimd.dma_start(x_t[:], src)  # SWDGE cast f32->fp16
                xt_t = xtpool.tile([P, EC, 4 * P], x_dt, tag="xt")
                if tmode == "matmul":
                    # regular matmul vs fp16 identity: exact transpose for
                    # fp16 data, HAM-warm, FWL; psum out must be f32 (1 bank
                    # per c-chunk), so copies are f32-sourced
                    for c in range(EC):
                        t_ps = ps_work.tile([P, 4 * P], F32, tag="ps")
                        for j in range(4):
                            nc.tensor.matmul(
                                t_ps[:, j * P:(j + 1) * P],
                                lhsT=x_t[:, j, c * P:(c + 1) * P],
                                rhs=ident[:],
                                start=True, stop=True)
                        copy_op(xt_t[:, c], t_ps[:],
                                period=(2 if tag == "q" else 3))
                else:
                    for c2 in range(EC // 2):
                        t_ps = ps_work.tile([P, 2, 4 * P], x_dt, tag="ps")
                        for cc in range(2):
                            c = 2 * c2 + cc
                            for j in range(4):
                                nc.tensor.transpose(
                                    t_ps[:, cc, j * P:(j + 1) * P],
                                    x_t[:, j, c * P:(c + 1) * P],
                                    ident)
                        # q sweep: ACT is idle early -> give it half the
                        # copies; k/v sweeps run while ACT does exp -> keep
                        # copies on DVE
                        copy_op(xt_t[:, 2 * c2:2 * c2 + 2], t_ps[:],
                                period=(2 if tag == "q" else 1000))
                # col-packed: pair A -> out partitions 0-63 (col grp 0),
                # pair B -> 64-127 (col grp 64); interleaved emission makes
                # them concurrent on HW. Separate PSUM tiles (banks) keep the
                # two accumulation groups' has_written state independent.
                psA = ps_work.tile([P, 2 * P], F32, tag="ps", name="projA")
                psB = ps_work.tile([P, 2 * P], F32, tag="ps", name="projB")
                w = w_sb[tag]
                for c in range(EC):
                    nc.tensor.matmul(
                        psA[0:H], lhsT=w[:, c], rhs=xt_t[:, c, 0:2 * P],
                        start=(c == 0), stop=(c == EC - 1))
                    nc.tensor.matmul(
                        psB[H:2 * H], lhsT=w[:, c],
                        rhs=xt_t[:, c, 2 * P:4 * P],
                        start=(c == 0), stop=(c == EC - 1))
                bias = b_sb[tag]
                slA = slice(s0, s0 + 2 * P)
                slB = slice(s0 + 2 * P, s0 + 4 * P)
                if tag == "v":
                    for sl_, src_ in ((slA, psA[0:H]), (slB, psB[H:2 * H])):
                        nc.vector.tensor_scalar(
                            out=vt_sb[:, sl_], in0=src_,
                            scalar1=bias, scalar2=None, op0=mybir.AluOpType.add)
                else:
                    dst = qt_sb if tag == "q" else kt_sb
                    for sl_, src_ in ((slA, psA[0:H]), (slB, psB[H:2 * H])):
                        nc.vector.tensor_scalar(
                            out=dst[0:H, sl_], in0=src_, scalar1=bias,
                            scalar2=None, op0=mybir.AluOpType.add)
                        # rowpack duplicate: fp16 SBUF->SBUF copy runs in
                        # DVE 4x mode
                        nc.vector.tensor_copy(out=dst[H:2 * H, sl_],
                                              in_=dst[0:H, sl_])

            def build_vprime(m):
                t_ps = ps_work.tile([P, H], F32, tag="ps")
                nc.tensor.transpose(
                    t_ps[:], vt_sb[:, m * P:(m + 1) * P],
                    ident_f32[:H, :H])
                nc.vector.tensor_copy(out=vp_sb[:, m, 0:H], in_=t_ps[:])

            def attend_kpair(p):
                """scores+exp for k-tiles m=2p, 2p+1 (row-packed).
                Interleave the two k-tiles so consecutive matmuls hit
                different PE row groups (concurrent on HW)."""
                ms = (2 * p, 2 * p + 1)
                sc = {}
                for idx, m in enumerate(ms):
                    sc[m] = ps_sc.tile([P, 2 * QCHUNK], F32, tag="sc",
                                       name=f"sc_{idx}")
                for qq in range(QC // 2):
                    for half in range(2):
                        qc = 2 * qq + half
                        for idx, m in enumerate(ms):
                            row0 = idx * H
                            nc.tensor.matmul(
                                sc[m][:, half * QCHUNK:(half + 1) * QCHUNK],
                                lhsT=kt_sb[row0:row0 + H, m * P:(m + 1) * P],
                                rhs=qt_sb[row0:row0 + H,
                                          qc * QCHUNK:(qc + 1) * QCHUNK],
                                start=True, stop=True)
                    for idx, m in enumerate(ms):
                        nc.scalar.activation(
                            exp_all[:, m, qq * 2 * QCHUNK:(qq + 1) * 2 * QCHUNK],
                            sc[m][:],
                            mybir.ActivationFunctionType.Exp, scale=0.125)

            # ---------------- phase 1a: full Xq sweep ----------------
            if do_full:
                for quad in range(NQUAD):
                    sweep_quad(xq_ext, quad, "q")

            outA_sb = persist.tile([H + 1, S], FP16, name="outA_sb")
            outA2_sb = persist.tile([H + 1, S], FP16, name="outA2_sb")

            def pv_half(qc, m_lo, m_hi, dst, accum_from=None):
                sl = slice(qc * QCHUNK, (qc + 1) * QCHUNK)
                pv_full = ps_work.tile([P, QCHUNK], F32, tag="ps")
                pv_ps = pv_full[:H + 1]
                for m in range(m_lo, m_hi):
                    nc.tensor.matmul(
                        pv_ps, lhsT=vp_sb[:, m],
                        rhs=exp_all[:, m, sl],
                        start=(m == m_lo), stop=(m == m_hi - 1))
                if accum_from is None:
                    copy_op(dst[:, sl], pv_ps)
                else:
                    nc.vector.tensor_tensor(
                        dst[:, sl], pv_ps, accum_from[:, sl],
                        mybir.AluOpType.add)

            # ------- phase 1b: K/V streams with attends interleaved -------
            if order == "ktv":
                # k before v: all scores/exp work is generated while the v
                # stream still has DMA to hide it under, and the last quad's
                # attends no longer land in the tail.
                for quad in range(NQUAD if do_full else 0):
                    sweep_quad(xk_ext, quad, "k")
                    if ablate in ("sweep", "noattend"):
                        continue
                    attend_kpair(2 * quad)
                    attend_kpair(2 * quad + 1)
                for quad in range(NQUAD if do_full else 0):
                    sweep_quad(xv_ext, quad, "v")
                    if ablate == "sweep":
                        continue
                    for t in range(4):
                        build_vprime(4 * quad + t)
                    if quad == 2 and ablate not in ("noattend", "nopv"):
                        # vp m0..7 ready after v-quad 1; exps all done in the
                        # k phase: first-half PV while v-quad 3 streams
                        for qc in range(QC):
                            pv_half(qc, 0, ST // 2, outA_sb)
            else:
                for quad in range(NQUAD if do_full else 0):
                    sweep_quad(xk_ext, quad, "k")
                    sweep_quad(xv_ext, quad, "v")
                    if ablate == "sweep":
                        continue
                    for t in range(4):
                        build_vprime(4 * quad + t)
                    if ablate == "noattend":
                        continue
                    attend_kpair(2 * quad)
                    attend_kpair(2 * quad + 1)
                    if quad == 2 and ablate != "nopv":
                        for qc in range(QC):
                            pv_half(qc, 0, 8, outA_sb)
                    if quad == 3 and ablate != "nopv":
                        # exps m8-11 (quad-2 attends) + vp m8-11 (v-quad 2)
                        # are ready; fold mid PV under the v-quad-3 DMA
                        for qc in range(QC):
                            pv_half(qc, 8, 12, outA2_sb, accum_from=outA_sb)

            # ------------- phase 3: PV-B + epilogue per q-chunk -------------
            outT_sb = persist.tile([H + 1, S], FP16, name="outT_sb")
            out_sb = persist.tile([P, ST, H], F32, name="out_sb")
            rc_sb = persist.tile([P, ST], F32, name="rc")
            for qc in range(QC if (do_full and ablate not in ("sweep", "noattend", "nopv")) else 0):
                sl = slice(qc * QCHUNK, (qc + 1) * QCHUNK)
                pv_half(qc, 12, ST, outT_sb, accum_from=outA2_sb)
                for mm in range(qc * QC, (qc + 1) * QC):
                    o_ps = ps_work.tile([P, H + 1], FP16, tag="ps")
                    nc.tensor.transpose(
                        o_ps[:], outT_sb[:, mm * P:(mm + 1) * P],
                        ident[:H + 1, :H + 1])
                    nc.vector.reciprocal(rc_sb[:, mm:mm + 1], o_ps[:, H:H + 1])
                    nc.vector.tensor_scalar(
                        out=out_sb[:, mm], in0=o_ps[:, 0:H],
                        scalar1=rc_sb[:, mm:mm + 1],
                        scalar2=None, op0=mybir.AluOpType.mult)
                if ablate != "noout":
                    nc.scalar.dma_start(
                        out_ext[qc * QC * P:(qc + 1) * QC * P, :].rearrange(
                            "(t p) h -> p t h", p=P),
                        out_sb[:, qc * QC:(qc + 1) * QC])

    nc.compile()
    return nc


_CACHE = {}


def _get_runner():
    if "runner" in _CACHE:
        return _CACHE["runner"]

    import functools
    import traceback

    import jax
    from jax.experimental.shard_map import shard_map
    from jax.sharding import Mesh, PartitionSpec

    from concourse import bass2jax
    from concourse.bass2jax import _bass_exec_p, partition_id_tensor

    bass2jax.install_neuronx_cc_hook()
    import libneuronxla
    hook = libneuronxla.neuronx_cc
    if not getattr(hook, "_verbose_wrapped", False):
        @functools.wraps(hook)
        def wrapped(*a, **k):
            try:
                return hook(*a, **k)
            except BaseException:
                traceback.print_exc()
                sys.stderr.flush()
                raise
        wrapped._verbose_wrapped = True
        libneuronxla.neuronx_cc = wrapped

    nc = build()

    partition_name = nc.partition_id_tensor.name if nc.partition_id_tensor else None
    in_names, out_names, out_avals, zero_outs = [], [], [], []
    for alloc in nc.m.functions[0].allocations:
        if not isinstance(alloc, mybir.MemoryLocationSet):
            continue
        name = alloc.memorylocations[0].name
        if alloc.kind == "ExternalInput":
            if name != partition_name:
                in_names.append(name)
        elif alloc.kind == "ExternalOutput":
            out_names.append(name)
            shape = tuple(alloc.tensor_shape)
            dtype = mybir.dt.np(alloc.dtype)
            out_avals.append(jax.core.ShapedArray(shape, dtype))
            zero_outs.append(np.zeros(shape, dtype))
    n_params = len(in_names)
    n_outs = len(out_avals)
    all_in_names = list(in_names) + out_names
    if partition_name is not None:
        all_in_names.append(partition_name)
    donate = tuple(range(n_params, n_params + n_outs))

    def _body(*args):
        operands = list(args)
        if partition_name is not None:
            operands.append(partition_id_tensor())
        outs = _bass_exec_p.bind(
            *operands,
            out_avals=tuple(out_avals),
            in_names=tuple(all_in_names),
            out_names=tuple(out_names),
            lowering_input_output_aliases=(),
            sim_require_finite=True,
            sim_require_nnan=True,
            nc=nc,
        )
        return tuple(outs)

    devices = jax.devices()[:B]
    mesh = Mesh(np.asarray(devices), ("core",))
    in_specs = (PartitionSpec("core"),) * (n_params + n_outs)
    out_specs = (PartitionSpec("core"),) * len(out_names)
    sharded = jax.jit(
        shard_map(_body, mesh=mesh, in_specs=in_specs,
                  out_specs=out_specs, check_rep=False),
        donate_argnums=donate, keep_unused=True)

    runner = {
        "sharded": sharded, "in_names": in_names, "out_names": out_names,
        "out_avals": out_avals, "zero_outs": zero_outs,
    }
    _CACHE["runner"] = runner
    return runner


def kernel(**inputs):
    r = _get_runner()
    per_core = {"query", "key", "value"}

    concat_in = []
    for name in r["in_names"]:
        arr = np.ascontiguousarray(np.asarray(inputs[name], dtype=np.float32))
        if name in per_core:
            concat_in.append(arr.reshape(B * S, E))
        else:
            concat_in.append(np.concatenate([arr] * B, axis=0))
    concat_zeros = [
        np.zeros((B * z.shape[0], *z.shape[1:]), z.dtype) for z in r["zero_outs"]
    ]
    out_arrs = r["sharded"](*concat_in, *concat_zeros)
    (aval,) = r["out_avals"]
    out = np.asarray(out_arrs[0]).reshape(B, *aval.shape)
    return out.astype(np.float32, copy=False)


if __name__ == "__main__":
    rng = np.random.default_rng(0)
    fake = {
        "query": rng.standard_normal((B, S, E), dtype=np.float32),
        "key": rng.standard_normal((B, S, E), dtype=np.float32),
        "value": rng.standard_normal((B, S, E), dtype=np.float32),
        "Wq": rng.standard_normal((E, H), dtype=np.float32) / 32,
        "bq": np.zeros(H, np.float32),
        "Wk": rng.standard_normal((E, H), dtype=np.float32) / 32,
        "bk": np.zeros(H, np.float32),
        "Wv": rng.standard_normal((E, H), dtype=np.float32) / 32,
        "bv": np.zeros(H, np.float32),
    }
    out = kernel(**fake)
    print("kernel out:", out.shape, out.dtype, float(out[0, 0, 0]))



# revision 5
# speedup vs baseline: 1.1675x; 1.0386x over previous
"""Trainium2 Bass kernel v2 for nn_Attention_67336497266780.

Single-head attention, B=8 S=2048 E=1024 H=64, data-parallel over batch:
each of the 8 NeuronCores computes one batch element end to end.

v2 changes vs baseline:
  - X path in fp16 (SWDGE cast-DMA from f32 DRAM): PE transposes at
    1 cyc/row (vs 2 for f32), DVE 2x-mode PSUM->SBUF copies, FWL on the
    projection weight loads.
  - Quad granularity (512 rows per DMA = 1 MB) for the input stream.
  - Projection col-packed 2x: two 64-row output chains at col groups
    0 and 64 of the PE array run concurrently on HW.
  - Score matmuls interleave the two row-packed k-tiles so consecutive
    matmuls target different PE row groups (concurrent on HW).
"""
import sys

sys.path.insert(0, "/opt/trn_rl_repo")

from contextlib import ExitStack

import numpy as np

import concourse.bass as bass
import concourse.mybir as mybir
import concourse.tile as tile
from concourse import bacc
from concourse.masks import make_identity

F32 = mybir.dt.float32
F32R = mybir.dt.float32r
FP16 = mybir.dt.float16

B = 8
P = 128
S = 2048
E = 1024
H = 64
EC = E // P          # 8 e-chunks
ST = S // P          # 16 s-tiles
NQUAD = ST // 4      # 4 quads (512 rows each)
QC = 4               # q chunks for scores/pv
QCHUNK = S // QC     # 512


def build(
    x_dt=FP16,
    scores_dt=FP16,
    pv_dt=FP16,
    debug=False,
    repeat=0,
    ablate="",   # "" | "dma" | "sweep" | "noattend" | "nopv"
    dma_engine="gpsimd",
    order="kv",      # "kv": q,(k,v)-interleaved | "ktv": q,k+attend,v+vprime
    tmode="transpose",  # "matmul": x-transposes as regular matmul vs identity
                     # (exact for fp16 data, HAM-warm, FWL); "transpose":
                     # PE transpose-mode
):
    nc = bacc.Bacc("TRN2", target_bir_lowering=False, debug=debug)

    xq_ext = nc.dram_tensor("query", [S, E], F32, kind="ExternalInput")
    xk_ext = nc.dram_tensor("key", [S, E], F32, kind="ExternalInput")
    xv_ext = nc.dram_tensor("value", [S, E], F32, kind="ExternalInput")
    wq_ext = nc.dram_tensor("Wq", [E, H], F32, kind="ExternalInput")
    wk_ext = nc.dram_tensor("Wk", [E, H], F32, kind="ExternalInput")
    wv_ext = nc.dram_tensor("Wv", [E, H], F32, kind="ExternalInput")
    bq_ext = nc.dram_tensor("bq", [H], F32, kind="ExternalInput")
    bk_ext = nc.dram_tensor("bk", [H], F32, kind="ExternalInput")
    bv_ext = nc.dram_tensor("bv", [H], F32, kind="ExternalInput")
    out_ext = nc.dram_tensor("out", [S, H], F32, kind="ExternalOutput")

    ctx = ExitStack()
    with tile.TileContext(nc) as tc, ctx:
        const = ctx.enter_context(tc.tile_pool(name="const", bufs=1))
        persist = ctx.enter_context(tc.tile_pool(name="persist", bufs=1))
        xpool = ctx.enter_context(tc.tile_pool(name="xpool", bufs=3))
        xtpool = ctx.enter_context(tc.tile_pool(name="xtpool", bufs=3))
        # transposes have their own psum pool: they alone release the x_t
        # DMA buffers, so they must never queue behind PV/epilogue psum use
        ps_tp = ctx.enter_context(tc.tile_pool(name="ps_tp", bufs=2, space="PSUM"))
        ps_work = ctx.enter_context(tc.tile_pool(name="ps_work", bufs=4, space="PSUM"))
        ps_sc = ctx.enter_context(tc.tile_pool(name="ps_sc", bufs=2, space="PSUM"))

        if repeat:
            hint = ((mybir.EngineType.Pool,) if ablate == "dma" else
                    (mybir.EngineType.PE, mybir.EngineType.DVE))
            loop_cm = tc.For_i(0, repeat, 1, hint_engines=hint)
        else:
            import contextlib
            loop_cm = contextlib.nullcontext()

        # ---- constants (outside timing loop) ----
        ident = const.tile([P, P], x_dt, name="ident")
        make_identity(nc, ident)
        ident_f32 = const.tile([P, P], F32, name="ident_f32")
        make_identity(nc, ident_f32)

        w_sb = {}
        b_sb = {}
        for name, wext, bext in (("q", wq_ext, bq_ext), ("k", wk_ext, bk_ext),
                                 ("v", wv_ext, bv_ext)):
            w = const.tile([P, EC, H], x_dt, name=f"w{name}")
            nc.gpsimd.dma_start(w[:], wext.rearrange("(o p) h -> p o h", p=P))
            w_sb[name] = w
            b = const.tile([H, 1], F32, name=f"b{name}")
            nc.scalar.dma_start(b[:], bext[:].unsqueeze(1))
            b_sb[name] = b

        # qt/kt duplicated across both partition halves for score rowpacking
        qt_sb = persist.tile([P, S], scores_dt, name="qt")
        kt_sb = persist.tile([P, S], scores_dt, name="kt")
        vt_sb = persist.tile([H, S], F32, name="vt")
        vp_sb = persist.tile([P, ST, H + 1], pv_dt, name="vprime")
        exp_all = persist.tile([P, ST, S], pv_dt, name="exp_all")

        copy_ctr = [0]

        def copy_op(out, in_, period=3):
            # fp16 copies go to DVE (2x perf mode); spread 1/period to ACT
            i = copy_ctr[0]
            copy_ctr[0] += 1
            if i % period == period - 1:
                nc.scalar.copy(out=out, in_=in_)
            else:
                nc.vector.tensor_copy(out=out, in_=in_)

        with loop_cm:
            if ablate == "dma":
                for quad in range(NQUAD):
                    for xext in (xq_ext, xk_ext, xv_ext):
                        s0 = quad * 4 * P
                        src = xext[s0:s0 + 4 * P, :].rearrange(
                            "(t p) e -> p t e", p=P)
                        if dma_engine == "gpsimd":
                            x_t = xpool.tile([P, 4, E], x_dt, tag="x")
                            nc.gpsimd.dma_start(x_t[:], src)
                        else:
                            x_t = xpool.tile([P, 4, E], F32, tag="xf")
                            nc.sync.dma_start(x_t[:], src)
            do_full = ablate != "dma"
            nc.vector.memset(vp_sb[:, :, H:H + 1], 1.0)

            def sweep_quad(xext, quad, tag):
                """Load + transpose + project 512 rows (4 s-tiles) of one
                input tensor. Projection col-packed 2x: pairs (A, B) of
                256 rows each accumulate into PSUM partitions 0-63 /
                64-127 concurrently."""
                s0 = quad * 4 * P
                x_t = xpool.tile([P, 4, E], x_dt, tag="x")
                src = xext[s0:s0 + 4 * P, :].rearrange("(t p) e -> p t e", p=P)
                nc.gpsimd.dma_start(x_t[:], src)  # SWDGE cast f32->fp16
                xt_t = xtpool.tile([P, EC, 4 * P], x_dt, tag="xt")
                if tmode == "matmul":
                    # regular matmul vs fp16 identity: exact transpose for
                    # fp16 data, HAM-warm, FWL; psum out must be f32 (1 bank
                    # per c-chunk), so copies are f32-sourced
                    for c in range(EC):
                        t_ps = ps_tp.tile([P, 4 * P], F32, tag="ps")
                        for j in range(4):
                            nc.tensor.matmul(
                                t_ps[:, j * P:(j + 1) * P],
                                lhsT=x_t[:, j, c * P:(c + 1) * P],
                                rhs=ident[:],
                                start=True, stop=True)
                        copy_op(xt_t[:, c], t_ps[:],
                                period=(2 if tag == "q" else 3))
                else:
                    for c2 in range(EC // 2):
                        t_ps = ps_tp.tile([P, 2, 4 * P], x_dt, tag="ps")
                        for cc in range(2):
                            c = 2 * c2 + cc
                            for j in range(4):
                                nc.tensor.transpose(
                                    t_ps[:, cc, j * P:(j + 1) * P],
                                    x_t[:, j, c * P:(c + 1) * P],
                                    ident)
                        # q sweep: ACT is idle early -> give it half the
                        # copies; k/v sweeps run while ACT does exp -> keep
                        # copies on DVE
                        copy_op(xt_t[:, 2 * c2:2 * c2 + 2], t_ps[:],
                                period=(2 if tag == "q" else 1000))
                # col-packed: pair A -> out partitions 0-63 (col grp 0),
                # pair B -> 64-127 (col grp 64); interleaved emission makes
                # them concurrent on HW. Separate PSUM tiles (banks) keep the
                # two accumulation groups' has_written state independent.
                psA = ps_work.tile([P, 2 * P], F32, tag="ps", name="projA")
                psB = ps_work.tile([P, 2 * P], F32, tag="ps", name="projB")
                w = w_sb[tag]
                for c in range(EC):
                    nc.tensor.matmul(
                        psA[0:H], lhsT=w[:, c], rhs=xt_t[:, c, 0:2 * P],
                        start=(c == 0), stop=(c == EC - 1))
                    nc.tensor.matmul(
                        psB[H:2 * H], lhsT=w[:, c],
                        rhs=xt_t[:, c, 2 * P:4 * P],
                        start=(c == 0), stop=(c == EC - 1))
                bias = b_sb[tag]
                slA = slice(s0, s0 + 2 * P)
                slB = slice(s0 + 2 * P, s0 + 4 * P)
                if tag == "v":
                    for sl_, src_ in ((slA, psA[0:H]), (slB, psB[H:2 * H])):
                        nc.vector.tensor_scalar(
                            out=vt_sb[:, sl_], in0=src_,
                            scalar1=bias, scalar2=None, op0=mybir.AluOpType.add)
                else:
                    dst = qt_sb if tag == "q" else kt_sb
                    for sl_, src_ in ((slA, psA[0:H]), (slB, psB[H:2 * H])):
                        nc.vector.tensor_scalar(
                            out=dst[0:H, sl_], in0=src_, scalar1=bias,
                            scalar2=None, op0=mybir.AluOpType.add)
                        # rowpack duplicate: fp16 SBUF->SBUF copy runs in
                        # DVE 4x mode
                        nc.vector.tensor_copy(out=dst[H:2 * H, sl_],
                                              in_=dst[0:H, sl_])

            def build_vprime(m):
                t_ps = ps_tp.tile([P, H], F32, tag="ps")
                nc.tensor.transpose(
                    t_ps[:], vt_sb[:, m * P:(m + 1) * P],
                    ident_f32[:H, :H])
                nc.vector.tensor_copy(out=vp_sb[:, m, 0:H], in_=t_ps[:])

            def attend_kpair(p):
                """scores+exp for k-tiles m=2p, 2p+1 (row-packed).
                Interleave the two k-tiles so consecutive matmuls hit
                different PE row groups (concurrent on HW)."""
                ms = (2 * p, 2 * p + 1)
                for qc in range(QC):
                    qsl = slice(qc * QCHUNK, (qc + 1) * QCHUNK)
                    sc = {}
                    for idx, m in enumerate(ms):
                        sc[m] = ps_sc.tile([P, QCHUNK], F32, tag="sc",
                                           name=f"sc_{idx}")
                    for idx, m in enumerate(ms):
                        row0 = idx * H
                        nc.tensor.matmul(
                            sc[m][:],
                            lhsT=kt_sb[row0:row0 + H, m * P:(m + 1) * P],
                            rhs=qt_sb[row0:row0 + H, qsl],
                            start=True, stop=True)
                    for idx, m in enumerate(ms):
                        nc.scalar.activation(
                            exp_all[:, m, qsl], sc[m][:],
                            mybir.ActivationFunctionType.Exp, scale=0.125)

            # ---------------- phase 1a: full Xq sweep ----------------
            if do_full:
                for quad in range(NQUAD):
                    sweep_quad(xq_ext, quad, "q")

            outA_sb = persist.tile([H + 1, S], FP16, name="outA_sb")
            outA2_sb = persist.tile([H + 1, S], FP16, name="outA2_sb")

            def pv_half(qc, m_lo, m_hi, dst, accum_from=None):
                sl = slice(qc * QCHUNK, (qc + 1) * QCHUNK)
                pv_full = ps_work.tile([P, QCHUNK], F32, tag="ps")
                pv_ps = pv_full[:H + 1]
                for m in range(m_lo, m_hi):
                    nc.tensor.matmul(
                        pv_ps, lhsT=vp_sb[:, m],
                        rhs=exp_all[:, m, sl],
                        start=(m == m_lo), stop=(m == m_hi - 1))
                if accum_from is None:
                    copy_op(dst[:, sl], pv_ps)
                else:
                    nc.vector.tensor_tensor(
                        dst[:, sl], pv_ps, accum_from[:, sl],
                        mybir.AluOpType.add)

            # ------- phase 1b: K/V streams with attends interleaved -------
            if order == "ktv":
                # k before v: all scores/exp work is generated while the v
                # stream still has DMA to hide it under, and the last quad's
                # attends no longer land in the tail.
                for quad in range(NQUAD if do_full else 0):
                    sweep_quad(xk_ext, quad, "k")
                    if ablate in ("sweep", "noattend"):
                        continue
                    attend_kpair(2 * quad)
                    attend_kpair(2 * quad + 1)
                for quad in range(NQUAD if do_full else 0):
                    sweep_quad(xv_ext, quad, "v")
                    if ablate == "sweep":
                        continue
                    for t in range(4):
                        build_vprime(4 * quad + t)
                    if quad == 2 and ablate not in ("noattend", "nopv"):
                        # vp m0..7 ready after v-quad 1; exps all done in the
                        # k phase: first-half PV while v-quad 3 streams
                        for qc in range(QC):
                            pv_half(qc, 0, ST // 2, outA_sb)
            else:
                for quad in range(NQUAD if do_full else 0):
                    sweep_quad(xk_ext, quad, "k")
                    sweep_quad(xv_ext, quad, "v")
                    if ablate == "sweep":
                        continue
                    for t in range(4):
                        build_vprime(4 * quad + t)
                    if ablate == "noattend":
                        continue
                    attend_kpair(2 * quad)
                    attend_kpair(2 * quad + 1)
                    if quad == 2 and ablate != "nopv":
                        for qc in range(QC):
                            pv_half(qc, 0, 8, outA_sb)
                    if quad == 3 and ablate != "nopv":
                        # exps m8-11 (quad-2 attends) + vp m8-11 (v-quad 2)
                        # are ready; fold mid PV under the v-quad-3 DMA
                        for qc in range(QC):
                            pv_half(qc, 8, 12, outA2_sb, accum_from=outA_sb)

            # ------------- phase 3: PV-B + epilogue per q-chunk -------------
            outT_sb = persist.tile([H + 1, S], FP16, name="outT_sb")
            out_sb = persist.tile([P, ST, H], F32, name="out_sb")
            rc_sb = persist.tile([P, ST], FP16, name="rc")
            for qc in range(QC if (do_full and ablate not in ("sweep", "noattend", "nopv")) else 0):
                sl = slice(qc * QCHUNK, (qc + 1) * QCHUNK)
                pv_half(qc, 12, ST, outT_sb, accum_from=outA2_sb)
                if ablate == "noepi":
                    continue
                # batched epilogue: 4 transposes into one psum tile, one
                # reciprocal over the 4 denominator columns, one broadcast
                # multiply -- short PE<->DVE dependency chain per q-chunk
                sl4 = slice(qc * QC, (qc + 1) * QC)
                o_ps = ps_work.tile([P, 4, H + 2], FP16, tag="ps")
                for t4 in range(4):
                    mm = qc * QC + t4
                    nc.tensor.transpose(
                        o_ps[:, t4, 0:H + 1], outT_sb[:, mm * P:(mm + 1) * P],
                        ident[:H + 1, :H + 1])
                with nc.allow_low_precision(
                        reason="1/denominator in fp16: denom is O(1e2-1e4), "
                               "rel err 2^-11 is far inside the 2e-2 gate"):
                    nc.vector.reciprocal(rc_sb[:, sl4], o_ps[:, :, H])
                nc.vector.tensor_tensor(
                    out_sb[:, sl4], o_ps[:, :, 0:H],
                    rc_sb[:, sl4].unsqueeze(2).broadcast_to([P, 4, H]),
                    mybir.AluOpType.mult)
                if ablate != "noout":
                    nc.scalar.dma_start(
                        out_ext[qc * QC * P:(qc + 1) * QC * P, :].rearrange(
                            "(t p) h -> p t h", p=P),
                        out_sb[:, qc * QC:(qc + 1) * QC])

    nc.compile()
    return nc


_CACHE = {}


def _get_runner():
    if "runner" in _CACHE:
        return _CACHE["runner"]

    import functools
    import traceback

    import jax
    from jax.experimental.shard_map import shard_map
    from jax.sharding import Mesh, PartitionSpec

    from concourse import bass2jax
    from concourse.bass2jax import _bass_exec_p, partition_id_tensor

    bass2jax.install_neuronx_cc_hook()
    import libneuronxla
    hook = libneuronxla.neuronx_cc
    if not getattr(hook, "_verbose_wrapped", False):
        @functools.wraps(hook)
        def wrapped(*a, **k):
            try:
                return hook(*a, **k)
            except BaseException:
                traceback.print_exc()
                sys.stderr.flush()
                raise
        wrapped._verbose_wrapped = True
        libneuronxla.neuronx_cc = wrapped

    nc = build()

    partition_name = nc.partition_id_tensor.name if nc.partition_id_tensor else None
    in_names, out_names, out_avals, zero_outs = [], [], [], []
    for alloc in nc.m.functions[0].allocations:
        if not isinstance(alloc, mybir.MemoryLocationSet):
            continue
        name = alloc.memorylocations[0].name
        if alloc.kind == "ExternalInput":
            if name != partition_name:
                in_names.append(name)
        elif alloc.kind == "ExternalOutput":
            out_names.append(name)
            shape = tuple(alloc.tensor_shape)
            dtype = mybir.dt.np(alloc.dtype)
            out_avals.append(jax.core.ShapedArray(shape, dtype))
            zero_outs.append(np.zeros(shape, dtype))
    n_params = len(in_names)
    n_outs = len(out_avals)
    all_in_names = list(in_names) + out_names
    if partition_name is not None:
        all_in_names.append(partition_name)
    donate = tuple(range(n_params, n_params + n_outs))

    def _body(*args):
        operands = list(args)
        if partition_name is not None:
            operands.append(partition_id_tensor())
        outs = _bass_exec_p.bind(
            *operands,
            out_avals=tuple(out_avals),
            in_names=tuple(all_in_names),
            out_names=tuple(out_names),
            lowering_input_output_aliases=(),
            sim_require_finite=True,
            sim_require_nnan=True,
            nc=nc,
        )
        return tuple(outs)

    devices = jax.devices()[:B]
    mesh = Mesh(np.asarray(devices), ("core",))
    in_specs = (PartitionSpec("core"),) * (n_params + n_outs)
    out_specs = (PartitionSpec("core"),) * len(out_names)
    sharded = jax.jit(
        shard_map(_body, mesh=mesh, in_specs=in_specs,
                  out_specs=out_specs, check_rep=False),
        donate_argnums=donate, keep_unused=True)

    runner = {
        "sharded": sharded, "in_names": in_names, "out_names": out_names,
        "out_avals": out_avals, "zero_outs": zero_outs,
    }
    _CACHE["runner"] = runner
    return runner


def kernel(**inputs):
    r = _get_runner()
    per_core = {"query", "key", "value"}

    concat_in = []
    for name in r["in_names"]:
        arr = np.ascontiguousarray(np.asarray(inputs[name], dtype=np.float32))
        if name in per_core:
            concat_in.append(arr.reshape(B * S, E))
        else:
            concat_in.append(np.concatenate([arr] * B, axis=0))
    concat_zeros = [
        np.zeros((B * z.shape[0], *z.shape[1:]), z.dtype) for z in r["zero_outs"]
    ]
    out_arrs = r["sharded"](*concat_in, *concat_zeros)
    (aval,) = r["out_avals"]
    out = np.asarray(out_arrs[0]).reshape(B, *aval.shape)
    return out.astype(np.float32, copy=False)


if __name__ == "__main__":
    rng = np.random.default_rng(0)
    fake = {
        "query": rng.standard_normal((B, S, E), dtype=np.float32),
        "key": rng.standard_normal((B, S, E), dtype=np.float32),
        "value": rng.standard_normal((B, S, E), dtype=np.float32),
        "Wq": rng.standard_normal((E, H), dtype=np.float32) / 32,
        "bq": np.zeros(H, np.float32),
        "Wk": rng.standard_normal((E, H), dtype=np.float32) / 32,
        "bk": np.zeros(H, np.float32),
        "Wv": rng.standard_normal((E, H), dtype=np.float32) / 32,
        "bv": np.zeros(H, np.float32),
    }
    out = kernel(**fake)
    print("kernel out:", out.shape, out.dtype, float(out[0, 0, 0]))

